# revision 37
# baseline (speedup 1.0000x reference)
"""Multi-head causal attention (QKV proj + attention + out proj) on 8 TRN2
NeuronCores.

Sharding: 2-way data-parallel over batch x 4-way tensor-parallel over heads
(Megatron-style).  Core c handles batch c//4 and heads [4*(c%4), 4*(c%4)+4).
Each core computes its 4 heads' Q/K/V projections (column-parallel), the
attention for those heads, and a partial output projection (row-parallel).
The host sums the 4 TP partials per batch and adds the output bias.

v2 design notes (vs the 333us baseline, which was PE-bound at 1.2 GHz):
  - everything fp16 on the wire: x, weights, output partials are cast on the
    host, halving HBM traffic (30MB -> ~15MB per core).  Host also
    pre-swizzles x/weights into partition-major layout so every input DMA is
    a contiguous per-partition stream.
  - exp batching: scores for 4 k-tiles accumulate into one [128, 2048] PSUM
    region (4 banks) and are exp'd by ONE activation instruction --
    (2048+352)/1.2 = 2.0us per 4 tiles vs 4x720ns.  ACT is the attention
    pace-setter so this matters.
  - normalization: reciprocal_approx_fast (custom DVE op, ~670ns for [1,512])
    replaces the 3355ns iterative reciprocal; the chain is emitted one head
    late so it never blocks the PE stream.
  - software-pipelined emission: the engines are in-order, so the PE
    instruction stream interleaves attention groups of chunk j with
    projection chains of chunk j+1 and out-projection blocks of chunk j-1
    ("filler" units).  The PE never sits on an exp dependency and the HAM
    clock gate stays at 8/8 (2.4 GHz).
"""

import numpy as np
from collections import deque
from contextlib import ExitStack

import concourse.bass as bass
import concourse.mybir as mybir
import concourse.tile as tile
from concourse import bacc
from concourse.bass import ds
from concourse.bass_utils import run_bass_kernel_spmd


B, S_FULL, E, H = 2, 2048, 1024, 16
D = E // H          # 64
NCORES = 8
TP = 4              # tensor-parallel ways (over heads)
HL = H // TP        # 4 local heads per core
F = HL * D          # 256 local projection width
P = 128
QCH = 512           # q-chunk / matmul moving-dim size
GRP = 4             # k-tiles exp'd per activation instruction
FP32 = mybir.dt.float32
F32R = mybir.dt.float32r
F16 = mybir.dt.float16
AF = mybir.ActivationFunctionType


def build(S=S_FULL, causal=True, debug=False):
    ET = E // P          # 8 contraction tiles for projections
    NQ = S // QCH        # 4 q chunks
    KT = S // P          # 16 k tiles
    KPQ = QCH // P       # 4 k tiles per q chunk

    nc = bacc.Bacc()

    def din(name, shape, dt=F16):
        return nc.declare_dram_parameter(name, shape, dt, isOutput=False)

    # host pre-swizzled, all fp16 (see make_in_maps)
    xq4 = din("xq4", [NQ, P, ET, QCH])
    xk4 = din("xk4", [NQ, P, ET, QCH])
    xv4 = din("xv4", [NQ, P, ET, QCH])
    wq3 = din("wq3", [P, ET, F])
    wk3 = din("wk3", [P, ET, F])
    wv3 = din("wv3", [P, ET, F])
    wo3 = din("wo3", [P, F // P, E])
    bcat = din("bcat", [P, 2 + 2 + F], FP32)   # bq2 | bk2 | bvb
    msk = din("msk", [P, KPQ, QCH])
    outT = nc.declare_dram_parameter("outT", [E, S], F16, isOutput=True)
    if debug:
        dbg_qT = nc.declare_dram_parameter("dbg_qT", [P, F // P, S], F16, isOutput=True)
        dbg_kT = nc.declare_dram_parameter("dbg_kT", [P, F // P, S], F16, isOutput=True)
        dbg_vo = nc.declare_dram_parameter("dbg_vo", [P, KT, HL, D + 1], F16, isOutput=True)
        dbg_oT = nc.declare_dram_parameter("dbg_oT", [P, F // P, S], F16, isOutput=True)
        dbg_po = nc.declare_dram_parameter("dbg_po", [P, HL, QCH], FP32, isOutput=True)
        dbg_bc = nc.declare_dram_parameter("dbg_bc", [D, HL, QCH], FP32, isOutput=True)

    with ExitStack() as ctx:
        ctx.enter_context(
            nc.allow_low_precision(reason="fp16 matmuls are the design point")
        )
        tc = ctx.enter_context(tile.TileContext(nc))
        const = ctx.enter_context(tc.tile_pool(name="const", bufs=1))
        xp = ctx.enter_context(tc.tile_pool(name="xp", bufs=2))  # 2 bufs x 3 tags
        ptp = ctx.enter_context(tc.tile_pool(name="ptp", bufs=2))
        dnp = ctx.enter_context(tc.tile_pool(name="dnp", bufs=2))
        otp = ctx.enter_context(tc.tile_pool(name="otp", bufs=3))
        # PSUM: sc 4 banks + acc 2 + po 2 = 8
        scp = ctx.enter_context(tc.tile_pool(name="scp", bufs=1, space="PSUM"))
        accp = ctx.enter_context(tc.tile_pool(name="accp", bufs=2, space="PSUM"))
        pop = ctx.enter_context(tc.tile_pool(name="pop", bufs=2, space="PSUM"))

        # ---- constants / persistent tensors ----
        # masks first: the PE warm-up matmuls depend only on this small DMA,
        # so the PE clock ramps while the big loads stream in.
        msk_sb = const.tile([P, KPQ, QCH], F16)
        nc.sync.dma_start(out=msk_sb, in_=msk[:, :, :])
        wq_sb = const.tile([P, ET, F], F16)
        nc.sync.dma_start(out=wq_sb, in_=wq3[:, :, :])
        wk_sb = const.tile([P, ET, F], F16)
        nc.sync.dma_start(out=wk_sb, in_=wk3[:, :, :])
        bcat_sb = const.tile([P, 2 + 2 + F], FP32)
        nc.sync.dma_start(out=bcat_sb, in_=bcat[:, :])
        bq_sb = bcat_sb[:, 0:2]
        bk_sb = bcat_sb[:, 2:4]
        bvb_sb = bcat_sb[:, 4:4 + F]
        # wv/wo load AFTER the chunk-0 activations (emitted in the main
        # sequence below): they aren't needed until the V projection /
        # out-projection, and this unblocks proj(0)'s QK chains ~3us earlier.
        wv_sb = const.tile([P, ET, F], F16)
        wo_sb = const.tile([P, F // P, E], F16)

        # PE clock warm-up: back-to-back dummy matmuls (WAW-serialized on the
        # acc pool) keep the tensor engine busy through the HAM window while
        # the input DMAs stream, so real work starts at 2.4 GHz.
        for _ in range(14):
            wps = accp.tile([P, QCH], FP32, tag="acc")
            nc.tensor.matmul(
                wps, msk_sb[:, 0, 0:P], msk_sb[:, 0, :], start=True, stop=True
            )

        ones_f32 = const.tile([P, D], FP32)
        nc.vector.memset(ones_f32, 1.0)
        ones_f16 = const.tile([P, D], F16)
        nc.scalar.activation(ones_f16, ones_f32, AF.Copy)

        qT_sb = const.tile([P, F // P, S], F16)
        kT_sb = const.tile([P, F // P, S], F16)
        # V with a trailing ones column: the PV matmul emits the softmax
        # denominator as PSUM row D for free.
        vo_sb = const.tile([P, KT, HL, D + 1], F16)
        nc.scalar.activation(
            vo_sb[:, :, :, D:D + 1],
            ones_f32[:, 0:KT * HL].rearrange("p (a b c) -> p a b c", a=KT, b=HL, c=1),
            AF.Copy,
        )
        oT_sb = const.tile([P, F // P, S], F16)
        # unnormalized attention output + denominator row (row D), per head
        ou_all = const.tile([P, HL, S], F16)

        # x chunk DMAs (j-granular); emitted early and prefetched one chunk
        # ahead by the main loop.
        x_tiles = {}

        def emit_x_dma(j):
            for name, src in (("q", xq4), ("k", xk4), ("v", xv4)):
                t = xp.tile([P, ET, QCH], F16, tag=f"x{name}")
                nc.sync.dma_start(out=t, in_=src[j])
                x_tiles[(name, j)] = t

        # ---- projection / out-projection unit generators (PE fillers) ----
        def proj_qk_unit(j, which, blk):
            xt = x_tiles[(which, j)]
            w_sb = wq_sb if which == "q" else wk_sb
            b_sb = bq_sb if which == "q" else bk_sb
            dst = qT_sb if which == "q" else kT_sb
            acc = accp.tile([P, QCH], FP32, tag="acc")
            for et in range(ET):
                nc.tensor.matmul(
                    acc,
                    w_sb[:, et, ds(blk * P, P)],
                    xt[:, et, :],
                    start=(et == 0),
                    stop=(et == ET - 1),
                )
            nc.vector.tensor_scalar_add(
                dst[:, blk, ds(j * QCH, QCH)], acc, b_sb[:, blk:blk + 1]
            )
            return 1750

        def proj_v_unit(j, sl):
            xt = x_tiles[("v", j)]
            st = j * KPQ + sl
            acc = accp.tile([P, QCH], FP32, tag="acc")
            for et in range(ET):
                nc.tensor.matmul(
                    acc[:, 0:F],
                    xt[:, et, ds(sl * P, P)],
                    wv_sb[:, et, :],
                    start=(et == 0),
                    stop=(et == ET - 1),
                )
            nc.vector.tensor_add(
                vo_sb[:, st, :, 0:D],
                acc[:, 0:F].rearrange("p (h d) -> p h d", h=HL),
                bvb_sb.rearrange("p (h d) -> p h d", h=HL),
            )
            return 900

        def outproj_unit(j, eb):
            acc = accp.tile([P, QCH], FP32, tag="acc")
            for fb in range(F // P):
                nc.tensor.matmul(
                    acc,
                    wo_sb[:, fb, ds(eb * P, P)],
                    oT_sb[:, fb, ds(j * QCH, QCH)],
                    start=(fb == 0),
                    stop=(fb == F // P - 1),
                )
            ot = otp.tile([P, QCH], F16, tag="ot")
            if eb % 2 == 0:
                nc.vector.tensor_copy(ot, acc)
            else:
                nc.scalar.activation(ot, acc, AF.Copy)
            nc.sync.dma_start(out=outT[ds(eb * P, P), ds(j * QCH, QCH)], in_=ot)
            return 500

        fillers = deque()

        def emit_dummy_mm():
            # keep-warm matmul: occupies an otherwise-idle PE stall window so
            # the HAM clock gate never sees an idle MID window (re-throttle
            # to 1.2 GHz costs far more than the dummy's 320ns).
            wps = accp.tile([P, QCH], FP32, tag="acc")
            nc.tensor.matmul(
                wps, msk_sb[:, 0, 0:P], msk_sb[:, 0, :], start=True, stop=True
            )
            return 320

        def do_filler(budget, pad=700):
            while budget > 0 and fillers:
                budget -= fillers.popleft()()
            pad = min(budget, pad)
            while pad > 0:
                pad -= emit_dummy_mm()

        def drain_fillers():
            while fillers:
                fillers.popleft()()

        def push_proj(j):
            for blk in range(F // P):
                fillers.append(lambda j=j, b=blk: proj_qk_unit(j, "q", b))
                fillers.append(lambda j=j, b=blk: proj_qk_unit(j, "k", b))
            for sl in range(KPQ):
                fillers.append(lambda j=j, s=sl: proj_v_unit(j, s))

        def push_outproj(j):
            for eb in range(E // P):
                fillers.append(lambda j=j, e=eb: outproj_unit(j, e))

        if debug:
            dbg_po_sb = const.tile([P, HL, QCH], FP32)
            dbg_bc_sb = const.tile([D, HL, QCH], FP32)

        # ---- normalization ----
        # po (unnormalized O + denom row) is evacuated to SBUF fp16 right
        # after the last PV; per chunk j, ONE tiny DMA reshapes the 4 heads'
        # denominator rows [1, 4x512] into [128, 16] so a single DVE
        # reciprocal covers them at full lane parallelism (~265ns vs 3.4us
        # per single-partition reciprocal), then a DMA puts 1/denom back as
        # a row for the PE broadcast matmuls.
        def emit_evac(j, h, po_t):
            nc.scalar.activation(
                ou_all[0:D + 1, h, ds(j * QCH, QCH)], po_t[0:D + 1, :], AF.Copy
            )
            if debug and j == 0:
                nc.vector.tensor_copy(dbg_po_sb[:, h, :], po_t)

        def emit_norm_pair(j, pr):
            PPH = QCH // 16  # 32 partitions per head's denominator row
            dn = dnp.tile([2 * PPH, 16], F16, tag="dn")
            for i in range(2):
                nc.sync.dma_start(
                    out=dn[i * PPH:(i + 1) * PPH, :],
                    in_=ou_all[D:D + 1, 2 * pr + i, ds(j * QCH, QCH)],
                )
            rc = dnp.tile([2 * PPH, 16], F16, tag="rc")
            nc.vector.reciprocal(rc, dn)
            rcr = dnp.tile([1, 2, QCH], F16, tag="rcr")
            for i in range(2):
                nc.sync.dma_start(
                    out=rcr[:, i, :], in_=rc[i * PPH:(i + 1) * PPH, :]
                )
            for i in range(2):
                h = 2 * pr + i
                doff = i * D
                bc = accp.tile([P, QCH], FP32, tag="acc")
                nc.tensor.matmul(
                    bc[0:D, :], ones_f16[0:1, :], rcr[:, i, :],
                    start=True, stop=True,
                )
                if debug and j == 0:
                    nc.vector.tensor_copy(dbg_bc_sb[:, h, :], bc[0:D, :])
                nc.vector.tensor_mul(
                    oT_sb[doff:doff + D, pr, ds(j * QCH, QCH)],
                    ou_all[0:D, h, ds(j * QCH, QCH)],
                    bc[0:D, :],
                )

        # ---- main emission loop ----
        emit_x_dma(0)
        nc.sync.dma_start(out=wv_sb, in_=wv3[:, :, :])
        nc.sync.dma_start(out=wo_sb, in_=wo3[:, :, :])
        push_proj(0)
        drain_fillers()          # projections for chunk 0 up front

        pending = None
        for j in range(NQ):
            if j + 1 < NQ:
                emit_x_dma(j + 1)
                push_proj(j + 1)
            for pr in range(HL // 2):
                # head pair (hA, hB) = (2*pr, 2*pr+1): hA's Q/K live on
                # partitions 0-63 of block pr, hB's on 64-127.  Their QK^T
                # matmuls (64-row contraction each) are emitted back-to-back
                # with explicit tile_position so they stream CONCURRENTLY
                # through disjoint PE row groups -- ~2x scores throughput.
                hA, hB = 2 * pr, 2 * pr + 1
                nkt = KPQ * (j + 1) if causal else KT
                ngrp = nkt // 2          # 2 k-tiles per head per group
                po_a = pop.tile([P, QCH], FP32, tag="po")
                po_b = pop.tile([P, QCH], FP32, tag="po")
                for g in range(ngrp):
                    kts = (2 * g, 2 * g + 1)
                    sc = scp.tile([P, GRP, QCH], FP32, tag="sc")
                    for i, kt in enumerate(kts):
                        nc.tensor.matmul(
                            sc[:, i, :],
                            kT_sb[0:D, pr, ds(kt * P, P)],
                            qT_sb[0:D, pr, ds(j * QCH, QCH)],
                            start=True, stop=True,
                            tile_position=(0, 0),
                        )
                        nc.tensor.matmul(
                            sc[:, 2 + i, :],
                            kT_sb[D:P, pr, ds(kt * P, P)],
                            qT_sb[D:P, pr, ds(j * QCH, QCH)],
                            start=True, stop=True,
                            tile_position=(64, 0),
                        )
                    pt = ptp.tile([P, GRP, QCH], F16, tag="pt")
                    is_diag = causal and kts[-1] >= KPQ * j
                    if j >= 2 and g % 2 == 0:
                        # late chunks are ACT-bound: alternate exp groups onto
                        # the DVE via the fp16 bit-trick
                        # exp(x) ~= bitcast_f16(int16(x*1024/ln2 + 15360)).
                        # Valid for x in (-10.4, 10.6); scores here are ~+-3.
                        nc.vector.tensor_scalar(
                            pt.bitcast(mybir.dt.int16),
                            sc,
                            1477.3194,
                            15360.0,
                            op0=mybir.AluOpType.mult,
                            op1=mybir.AluOpType.add,
                        )
                    else:
                        nc.scalar.activation(pt, sc, AF.Exp)
                    if is_diag:
                        # diagonal group: zero the upper-triangular part.
                        # GpSimd is otherwise idle and DVE is the late-chunk
                        # pace-setter, so the mask multiplies go there.
                        eng = nc.gpsimd if j >= 2 else nc.vector
                        toff = 2 * g - KPQ * j   # mask slot of kts[0]
                        eng.tensor_mul(
                            pt[:, 0:2, :], pt[:, 0:2, :],
                            msk_sb[:, toff:toff + 2, :],
                        )
                        eng.tensor_mul(
                            pt[:, 2:4, :], pt[:, 2:4, :],
                            msk_sb[:, toff:toff + 2, :],
                        )
                    do_filler(1400)
                    for i, kt in enumerate(kts):
                        nc.tensor.matmul(
                            po_a[0:D + 1, :],
                            vo_sb[:, kt, hA, :],
                            pt[:, i, :],
                            start=(kt == 0),
                            stop=(kt == nkt - 1),
                        )
                        nc.tensor.matmul(
                            po_b[0:D + 1, :],
                            vo_sb[:, kt, hB, :],
                            pt[:, 2 + i, :],
                            start=(kt == 0),
                            stop=(kt == nkt - 1),
                        )
                emit_evac(j, hA, po_a)
                emit_evac(j, hB, po_b)
                if pending is not None:
                    pj, ppr = pending
                    emit_norm_pair(pj, ppr)
                    if ppr == 1:
                        push_outproj(pj)
                    pending = None
                pending = (j, pr)
            # chunk boundary: everything for chunk j+1's attention must be
            # emitted before its first scores matmul.
            drain_fillers()
        emit_norm_pair(*pending)
        push_outproj(NQ - 1)
        drain_fillers()
        if debug:
            nc.sync.dma_start(out=dbg_qT[:, :, :], in_=qT_sb)
            nc.sync.dma_start(out=dbg_kT[:, :, :], in_=kT_sb)
            nc.sync.dma_start(out=dbg_vo[:, :, :, :], in_=vo_sb)
            nc.sync.dma_start(out=dbg_oT[:, :, :], in_=oT_sb)
            nc.sync.dma_start(out=dbg_po[:, :, :], in_=dbg_po_sb)
            nc.sync.dma_start(out=dbg_bc[:, :, :], in_=dbg_bc_sb)

    nc.compile()
    return nc


def make_masks(S=S_FULL):
    KPQ = QCH // P
    m = np.zeros((P, KPQ, QCH), np.float32)
    for t in range(KPQ):
        kk = np.arange(P)[:, None]
        qq = np.arange(QCH)[None, :]
        m[:, t, :] = (qq >= kk + P * t).astype(np.float32)
    return m


def make_in_maps(query, key, value, Wq, bq, Wk, bk, Wv, bv, Wo, bo, S=S_FULL):
    scale = float(D) ** -0.5
    ET = E // P
    NQ = S // QCH
    q = np.asarray(query, np.float32)
    k = np.asarray(key, np.float32)
    v = np.asarray(value, np.float32)
    Wq = np.asarray(Wq, np.float32)
    Wk = np.asarray(Wk, np.float32)
    Wv = np.asarray(Wv, np.float32)
    Wo = np.asarray(Wo, np.float32)
    bq = np.asarray(bq, np.float32)
    bk = np.asarray(bk, np.float32)
    bv = np.asarray(bv, np.float32)

    def xswiz(xT):
        # [E, S] -> [NQ, P, ET, QCH]: contiguous per-partition DMA streams
        return np.ascontiguousarray(
            xT.reshape(ET, P, NQ, QCH).transpose(2, 1, 0, 3).astype(np.float16)
        )

    def wswiz(wT):
        # [E, F] -> [P, ET, F]
        return np.ascontiguousarray(
            wT.reshape(ET, P, F).transpose(1, 0, 2).astype(np.float16)
        )

    masks = make_masks(S).astype(np.float16)
    in_maps = []
    for c in range(NCORES):
        b, tp = divmod(c, TP)
        rows = slice(tp * F, (tp + 1) * F)
        bq2 = (bq[rows] * scale).reshape(F // P, P).T        # [P, 2]
        bk2 = bk[rows].reshape(F // P, P).T                  # [P, 2]
        bvb = np.broadcast_to(bv[rows], (P, F))              # [P, F]
        bcat = np.concatenate([bq2, bk2, bvb], axis=1).astype(np.float32)
        woT = Wo[:, rows].T                                  # [F, E]
        wo3 = woT.reshape(F // P, P, E).transpose(1, 0, 2).astype(np.float16)
        in_maps.append({
            "xq4": xswiz(q[b].T),
            "xk4": xswiz(k[b].T),
            "xv4": xswiz(v[b].T),
            "wq3": wswiz((Wq[rows] * scale).T),
            "wk3": wswiz(Wk[rows].T),
            "wv3": wswiz(Wv[rows].T),
            "wo3": np.ascontiguousarray(wo3),
            "bcat": np.ascontiguousarray(bcat),
            "msk": masks,
        })
    return in_maps


_CACHE = {}


def _get_nc(causal):
    if causal not in _CACHE:
        _CACHE[causal] = build(S_FULL, causal)
    return _CACHE[causal]


def kernel(query, key, value, Wq, bq, Wk, bk, Wv, bv, Wo, bo, is_causal):
    causal = bool(int(np.asarray(is_causal)))
    nc = _get_nc(causal)
    in_maps = make_in_maps(query, key, value, Wq, bq, Wk, bk, Wv, bv, Wo, bo)
    res = run_bass_kernel_spmd(nc, in_maps, core_ids=list(range(NCORES)))
    out = np.zeros((B, S_FULL, E), np.float32)
    for c in range(NCORES):
        b, tp = divmod(c, TP)
        out[b] += res.results[c]["outT"].T.astype(np.float32)
    out += np.asarray(bo, np.float32)
    return out


# revision 39
# speedup vs baseline: 1.2393x; 1.2393x over previous
"""Multi-head causal attention (QKV proj + attention + out proj) on 8 TRN2
NeuronCores.

Sharding: 2-way data-parallel over batch x 4-way tensor-parallel over heads
(Megatron-style).  Core c handles batch c//4 and heads [4*(c%4), 4*(c%4)+4).
Each core computes its 4 heads' Q/K/V projections (column-parallel), the
attention for those heads, and a partial output projection (row-parallel).
The host sums the 4 TP partials per batch and adds the output bias.

v2 design notes (vs the 333us baseline, which was PE-bound at 1.2 GHz):
  - everything fp16 on the wire: x, weights, output partials are cast on the
    host, halving HBM traffic (30MB -> ~15MB per core).  Host also
    pre-swizzles x/weights into partition-major layout so every input DMA is
    a contiguous per-partition stream.
  - exp batching: scores for 4 k-tiles accumulate into one [128, 2048] PSUM
    region (4 banks) and are exp'd by ONE activation instruction --
    (2048+352)/1.2 = 2.0us per 4 tiles vs 4x720ns.  ACT is the attention
    pace-setter so this matters.
  - normalization: reciprocal_approx_fast (custom DVE op, ~670ns for [1,512])
    replaces the 3355ns iterative reciprocal; the chain is emitted one head
    late so it never blocks the PE stream.
  - software-pipelined emission: the engines are in-order, so the PE
    instruction stream interleaves attention groups of chunk j with
    projection chains of chunk j+1 and out-projection blocks of chunk j-1
    ("filler" units).  The PE never sits on an exp dependency and the HAM
    clock gate stays at 8/8 (2.4 GHz).
"""

import numpy as np
from collections import deque
from contextlib import ExitStack

import concourse.bass as bass
import concourse.mybir as mybir
import concourse.tile as tile
from concourse import bacc
from concourse.bass import ds
from concourse.bass_utils import run_bass_kernel_spmd


B, S_FULL, E, H = 2, 2048, 1024, 16
D = E // H          # 64
NCORES = 8
TP = 4              # tensor-parallel ways (over heads)
HL = H // TP        # 4 local heads per core
F = HL * D          # 256 local projection width
P = 128
QCH = 512           # q-chunk / matmul moving-dim size
GRP = 4             # k-tiles exp'd per activation instruction
FP32 = mybir.dt.float32
F32R = mybir.dt.float32r
F16 = mybir.dt.float16
AF = mybir.ActivationFunctionType


def build(S=S_FULL, causal=True, debug=False):
    ET = E // P          # 8 contraction tiles for projections
    NQ = S // QCH        # 4 q chunks
    KT = S // P          # 16 k tiles
    KPQ = QCH // P       # 4 k tiles per q chunk

    nc = bacc.Bacc()

    def din(name, shape, dt=F16):
        return nc.declare_dram_parameter(name, shape, dt, isOutput=False)

    # host pre-swizzled, all fp16 (see make_in_maps)
    xq4 = din("xq4", [NQ, P, ET, QCH])
    xk4 = din("xk4", [NQ, P, ET, QCH])
    xv4 = din("xv4", [NQ, P, ET, QCH])
    wq3 = din("wq3", [P, ET, F])
    wk3 = din("wk3", [P, ET, F])
    wv3 = din("wv3", [P, ET, F])
    wo3 = din("wo3", [P, F // P, E])
    bcat = din("bcat", [P, 2 + 2 + F], FP32)   # bq2 | bk2 | bvb
    msk = din("msk", [P, KPQ, QCH])
    outT = nc.declare_dram_parameter("outT", [E, S], F16, isOutput=True)
    if debug:
        dbg_qT = nc.declare_dram_parameter("dbg_qT", [P, F // P, S], F16, isOutput=True)
        dbg_kT = nc.declare_dram_parameter("dbg_kT", [P, F // P, S], F16, isOutput=True)
        dbg_vo = nc.declare_dram_parameter("dbg_vo", [P, KT, HL, D + 1], F16, isOutput=True)
        dbg_oT = nc.declare_dram_parameter("dbg_oT", [P, F // P, S], F16, isOutput=True)
        dbg_po = nc.declare_dram_parameter("dbg_po", [P, HL, QCH], FP32, isOutput=True)
        dbg_bc = nc.declare_dram_parameter("dbg_bc", [D, HL, QCH], FP32, isOutput=True)

    with ExitStack() as ctx:
        ctx.enter_context(
            nc.allow_low_precision(reason="fp16 matmuls are the design point")
        )
        tc = ctx.enter_context(tile.TileContext(nc))
        const = ctx.enter_context(tc.tile_pool(name="const", bufs=1))
        xp = ctx.enter_context(tc.tile_pool(name="xp", bufs=2))  # 2 bufs x 3 tags
        ptp = ctx.enter_context(tc.tile_pool(name="ptp", bufs=2))
        dnp = ctx.enter_context(tc.tile_pool(name="dnp", bufs=2))
        otp = ctx.enter_context(tc.tile_pool(name="otp", bufs=3))
        # PSUM: sc 4 banks + acc 2 + po 2 = 8
        scp = ctx.enter_context(tc.tile_pool(name="scp", bufs=1, space="PSUM"))
        accp = ctx.enter_context(tc.tile_pool(name="accp", bufs=2, space="PSUM"))
        pop = ctx.enter_context(tc.tile_pool(name="pop", bufs=2, space="PSUM"))

        # ---- constants / persistent tensors ----
        # masks first: the PE warm-up matmuls depend only on this small DMA,
        # so the PE clock ramps while the big loads stream in.
        msk_sb = const.tile([P, KPQ, QCH], F16)
        nc.sync.dma_start(out=msk_sb, in_=msk[:, :, :])
        wq_sb = const.tile([P, ET, F], F16)
        nc.sync.dma_start(out=wq_sb, in_=wq3[:, :, :])
        wk_sb = const.tile([P, ET, F], F16)
        nc.sync.dma_start(out=wk_sb, in_=wk3[:, :, :])
        bcat_sb = const.tile([P, 2 + 2 + F], FP32)
        nc.sync.dma_start(out=bcat_sb, in_=bcat[:, :])
        bq_sb = bcat_sb[:, 0:2]
        bk_sb = bcat_sb[:, 2:4]
        bvb_sb = bcat_sb[:, 4:4 + F]
        # wv/wo load AFTER the chunk-0 activations (emitted in the main
        # sequence below): they aren't needed until the V projection /
        # out-projection, and this unblocks proj(0)'s QK chains ~3us earlier.
        wv_sb = const.tile([P, ET, F], F16)
        wo_sb = const.tile([P, F // P, E], F16)

        # PE clock warm-up: back-to-back dummy matmuls (WAW-serialized on the
        # acc pool) keep the tensor engine busy through the HAM window while
        # the input DMAs stream, so real work starts at 2.4 GHz.
        for _ in range(14):
            wps = accp.tile([P, QCH], FP32, tag="acc")
            nc.tensor.matmul(
                wps, msk_sb[:, 0, 0:P], msk_sb[:, 0, :], start=True, stop=True
            )

        ones_f32 = const.tile([P, D], FP32)
        nc.vector.memset(ones_f32, 1.0)
        ones_f16 = const.tile([P, D], F16)
        nc.scalar.activation(ones_f16, ones_f32, AF.Copy)

        qT_sb = const.tile([P, F // P, S], F16)
        kT_sb = const.tile([P, F // P, S], F16)
        # V with a trailing ones column: the PV matmul emits the softmax
        # denominator as PSUM row D for free.
        vo_sb = const.tile([P, KT, HL, D + 1], F16)
        nc.scalar.activation(
            vo_sb[:, :, :, D:D + 1],
            ones_f32[:, 0:KT * HL].rearrange("p (a b c) -> p a b c", a=KT, b=HL, c=1),
            AF.Copy,
        )
        oT_sb = const.tile([P, F // P, S], F16)
        # unnormalized attention output + denominator row (row D), per head
        ou_all = const.tile([P, HL, S], F16)

        # x chunk DMAs (j-granular); emitted early and prefetched one chunk
        # ahead by the main loop.
        x_tiles = {}

        def emit_x_dma(j):
            for name, src in (("q", xq4), ("k", xk4), ("v", xv4)):
                t = xp.tile([P, ET, QCH], F16, tag=f"x{name}")
                nc.sync.dma_start(out=t, in_=src[j])
                x_tiles[(name, j)] = t

        # ---- projection / out-projection unit generators (PE fillers) ----
        def proj_qk_unit(j, which, blk):
            xt = x_tiles[(which, j)]
            w_sb = wq_sb if which == "q" else wk_sb
            b_sb = bq_sb if which == "q" else bk_sb
            dst = qT_sb if which == "q" else kT_sb
            acc = accp.tile([P, QCH], FP32, tag="acc")
            for et in range(ET):
                nc.tensor.matmul(
                    acc,
                    w_sb[:, et, ds(blk * P, P)],
                    xt[:, et, :],
                    start=(et == 0),
                    stop=(et == ET - 1),
                )
            nc.vector.tensor_scalar_add(
                dst[:, blk, ds(j * QCH, QCH)], acc, b_sb[:, blk:blk + 1]
            )
            return 1750

        def proj_v_unit(j, sl):
            xt = x_tiles[("v", j)]
            st = j * KPQ + sl
            acc = accp.tile([P, QCH], FP32, tag="acc")
            for et in range(ET):
                nc.tensor.matmul(
                    acc[:, 0:F],
                    xt[:, et, ds(sl * P, P)],
                    wv_sb[:, et, :],
                    start=(et == 0),
                    stop=(et == ET - 1),
                )
            nc.vector.tensor_add(
                vo_sb[:, st, :, 0:D],
                acc[:, 0:F].rearrange("p (h d) -> p h d", h=HL),
                bvb_sb.rearrange("p (h d) -> p h d", h=HL),
            )
            return 900

        def outproj_unit(j, eb):
            acc = accp.tile([P, QCH], FP32, tag="acc")
            for fb in range(F // P):
                nc.tensor.matmul(
                    acc,
                    wo_sb[:, fb, ds(eb * P, P)],
                    oT_sb[:, fb, ds(j * QCH, QCH)],
                    start=(fb == 0),
                    stop=(fb == F // P - 1),
                )
            ot = otp.tile([P, QCH], F16, tag="ot")
            if eb % 2 == 0:
                nc.vector.tensor_copy(ot, acc)
            else:
                nc.scalar.activation(ot, acc, AF.Copy)
            nc.sync.dma_start(out=outT[ds(eb * P, P), ds(j * QCH, QCH)], in_=ot)
            return 500

        fillers = deque()

        def emit_dummy_mm():
            # keep-warm matmul: occupies an otherwise-idle PE stall window so
            # the HAM clock gate never sees an idle MID window (re-throttle
            # to 1.2 GHz costs far more than the dummy's 320ns).
            wps = accp.tile([P, QCH], FP32, tag="acc")
            nc.tensor.matmul(
                wps, msk_sb[:, 0, 0:P], msk_sb[:, 0, :], start=True, stop=True
            )
            return 320

        def do_filler(budget, pad=700):
            while budget > 0 and fillers:
                budget -= fillers.popleft()()
            pad = min(budget, pad)
            while pad > 0:
                pad -= emit_dummy_mm()

        def drain_fillers():
            while fillers:
                fillers.popleft()()

        def push_proj(j):
            for blk in range(F // P):
                fillers.append(lambda j=j, b=blk: proj_qk_unit(j, "q", b))
                fillers.append(lambda j=j, b=blk: proj_qk_unit(j, "k", b))
            for sl in range(KPQ):
                fillers.append(lambda j=j, s=sl: proj_v_unit(j, s))

        def push_outproj(j):
            for eb in range(E // P):
                fillers.append(lambda j=j, e=eb: outproj_unit(j, e))

        if debug:
            dbg_po_sb = const.tile([P, HL, QCH], FP32)
            dbg_bc_sb = const.tile([D, HL, QCH], FP32)

        # ---- normalization ----
        # po (unnormalized O + denom row) is evacuated to SBUF fp16 right
        # after the last PV; per chunk j, ONE tiny DMA reshapes the 4 heads'
        # denominator rows [1, 4x512] into [128, 16] so a single DVE
        # reciprocal covers them at full lane parallelism (~265ns vs 3.4us
        # per single-partition reciprocal), then a DMA puts 1/denom back as
        # a row for the PE broadcast matmuls.
        def emit_evac(j, h, po_t):
            nc.scalar.activation(
                ou_all[0:D + 1, h, ds(j * QCH, QCH)], po_t[0:D + 1, :], AF.Copy
            )
            if debug and j == 0:
                nc.vector.tensor_copy(dbg_po_sb[:, h, :], po_t)

        def emit_norm_pair(j, pr):
            PPH = QCH // 16  # 32 partitions per head's denominator row
            dn = dnp.tile([2 * PPH, 16], F16, tag="dn")
            for i in range(2):
                nc.sync.dma_start(
                    out=dn[i * PPH:(i + 1) * PPH, :],
                    in_=ou_all[D:D + 1, 2 * pr + i, ds(j * QCH, QCH)],
                )
            rc = dnp.tile([2 * PPH, 16], F16, tag="rc")
            nc.vector.reciprocal(rc, dn)
            rcr = dnp.tile([1, 2, QCH], F16, tag="rcr")
            for i in range(2):
                nc.sync.dma_start(
                    out=rcr[:, i, :], in_=rc[i * PPH:(i + 1) * PPH, :]
                )
            for i in range(2):
                h = 2 * pr + i
                doff = i * D
                bc = accp.tile([P, QCH], FP32, tag="acc")
                nc.tensor.matmul(
                    bc[0:D, :], ones_f16[0:1, :], rcr[:, i, :],
                    start=True, stop=True,
                )
                if debug and j == 0:
                    nc.vector.tensor_copy(dbg_bc_sb[:, h, :], bc[0:D, :])
                nc.vector.tensor_mul(
                    oT_sb[doff:doff + D, pr, ds(j * QCH, QCH)],
                    ou_all[0:D, h, ds(j * QCH, QCH)],
                    bc[0:D, :],
                )

        # ---- main emission loop ----
        emit_x_dma(0)
        nc.sync.dma_start(out=wv_sb, in_=wv3[:, :, :])
        nc.sync.dma_start(out=wo_sb, in_=wo3[:, :, :])
        push_proj(0)
        drain_fillers()          # projections for chunk 0 up front

        pending = None
        for j in range(NQ):
            if j + 1 < NQ:
                emit_x_dma(j + 1)
                push_proj(j + 1)
            for pr in range(HL // 2):
                # head pair (hA, hB) = (2*pr, 2*pr+1): hA's Q/K live on
                # partitions 0-63 of block pr, hB's on 64-127.  Their QK^T
                # matmuls (64-row contraction each) are emitted back-to-back
                # with explicit tile_position so they stream CONCURRENTLY
                # through disjoint PE row groups -- ~2x scores throughput.
                hA, hB = 2 * pr, 2 * pr + 1
                nkt = KPQ * (j + 1) if causal else KT
                ngrp = nkt // 2          # 2 k-tiles per head per group
                po_a = pop.tile([P, QCH], FP32, tag="po")
                po_b = pop.tile([P, QCH], FP32, tag="po")
                for g in range(ngrp):
                    kts = (2 * g, 2 * g + 1)
                    sc = scp.tile([P, GRP, QCH], FP32, tag="sc")
                    for i, kt in enumerate(kts):
                        nc.tensor.matmul(
                            sc[:, i, :],
                            kT_sb[0:D, pr, ds(kt * P, P)],
                            qT_sb[0:D, pr, ds(j * QCH, QCH)],
                            start=True, stop=True,
                            tile_position=(0, 0),
                        )
                        nc.tensor.matmul(
                            sc[:, 2 + i, :],
                            kT_sb[D:P, pr, ds(kt * P, P)],
                            qT_sb[D:P, pr, ds(j * QCH, QCH)],
                            start=True, stop=True,
                            tile_position=(64, 0),
                        )
                    pt = ptp.tile([P, GRP, QCH], F16, tag="pt")
                    is_diag = causal and kts[-1] >= KPQ * j
                    if j >= 2 and g % 3 == 0:
                        # late chunks are ACT-bound: alternate exp groups onto
                        # the DVE via the fp16 bit-trick
                        # exp(x) ~= bitcast_f16(int16(x*1024/ln2 + 15360)).
                        # Valid for x in (-10.4, 10.6); scores here are ~+-3.
                        nc.vector.tensor_scalar(
                            pt.bitcast(mybir.dt.int16),
                            sc,
                            1477.3194,
                            15360.0,
                            op0=mybir.AluOpType.mult,
                            op1=mybir.AluOpType.add,
                        )
                    else:
                        nc.scalar.activation(pt, sc, AF.Exp)
                    if is_diag:
                        # diagonal group: zero the upper-triangular part
                        # (gpsimd TTs measured 2.5us each -- stay on DVE)
                        eng = nc.vector
                        toff = 2 * g - KPQ * j   # mask slot of kts[0]
                        eng.tensor_mul(
                            pt[:, 0:2, :], pt[:, 0:2, :],
                            msk_sb[:, toff:toff + 2, :],
                        )
                        eng.tensor_mul(
                            pt[:, 2:4, :], pt[:, 2:4, :],
                            msk_sb[:, toff:toff + 2, :],
                        )
                    do_filler(1400)
                    for i, kt in enumerate(kts):
                        nc.tensor.matmul(
                            po_a[0:D + 1, :],
                            vo_sb[:, kt, hA, :],
                            pt[:, i, :],
                            start=(kt == 0),
                            stop=(kt == nkt - 1),
                        )
                        nc.tensor.matmul(
                            po_b[0:D + 1, :],
                            vo_sb[:, kt, hB, :],
                            pt[:, 2 + i, :],
                            start=(kt == 0),
                            stop=(kt == nkt - 1),
                        )
                emit_evac(j, hA, po_a)
                emit_evac(j, hB, po_b)
                if pending is not None:
                    pj, ppr = pending
                    emit_norm_pair(pj, ppr)
                    if ppr == 1:
                        push_outproj(pj)
                    pending = None
                pending = (j, pr)
            # chunk boundary: everything for chunk j+1's attention must be
            # emitted before its first scores matmul.
            drain_fillers()
        emit_norm_pair(*pending)
        push_outproj(NQ - 1)
        drain_fillers()
        if debug:
            nc.sync.dma_start(out=dbg_qT[:, :, :], in_=qT_sb)
            nc.sync.dma_start(out=dbg_kT[:, :, :], in_=kT_sb)
            nc.sync.dma_start(out=dbg_vo[:, :, :, :], in_=vo_sb)
            nc.sync.dma_start(out=dbg_oT[:, :, :], in_=oT_sb)
            nc.sync.dma_start(out=dbg_po[:, :, :], in_=dbg_po_sb)
            nc.sync.dma_start(out=dbg_bc[:, :, :], in_=dbg_bc_sb)

    nc.compile()
    return nc


def make_masks(S=S_FULL):
    KPQ = QCH // P
    m = np.zeros((P, KPQ, QCH), np.float32)
    for t in range(KPQ):
        kk = np.arange(P)[:, None]
        qq = np.arange(QCH)[None, :]
        m[:, t, :] = (qq >= kk + P * t).astype(np.float32)
    return m


def make_in_maps(query, key, value, Wq, bq, Wk, bk, Wv, bv, Wo, bo, S=S_FULL):
    scale = float(D) ** -0.5
    ET = E // P
    NQ = S // QCH
    q = np.asarray(query, np.float32)
    k = np.asarray(key, np.float32)
    v = np.asarray(value, np.float32)
    Wq = np.asarray(Wq, np.float32)
    Wk = np.asarray(Wk, np.float32)
    Wv = np.asarray(Wv, np.float32)
    Wo = np.asarray(Wo, np.float32)
    bq = np.asarray(bq, np.float32)
    bk = np.asarray(bk, np.float32)
    bv = np.asarray(bv, np.float32)

    def xswiz(xT):
        # [E, S] -> [NQ, P, ET, QCH]: contiguous per-partition DMA streams
        return np.ascontiguousarray(
            xT.reshape(ET, P, NQ, QCH).transpose(2, 1, 0, 3).astype(np.float16)
        )

    def wswiz(wT):
        # [E, F] -> [P, ET, F]
        return np.ascontiguousarray(
            wT.reshape(ET, P, F).transpose(1, 0, 2).astype(np.float16)
        )

    masks = make_masks(S).astype(np.float16)
    in_maps = []
    for c in range(NCORES):
        b, tp = divmod(c, TP)
        rows = slice(tp * F, (tp + 1) * F)
        bq2 = (bq[rows] * scale).reshape(F // P, P).T        # [P, 2]
        bk2 = bk[rows].reshape(F // P, P).T                  # [P, 2]
        bvb = np.broadcast_to(bv[rows], (P, F))              # [P, F]
        bcat = np.concatenate([bq2, bk2, bvb], axis=1).astype(np.float32)
        woT = Wo[:, rows].T                                  # [F, E]
        wo3 = woT.reshape(F // P, P, E).transpose(1, 0, 2).astype(np.float16)
        in_maps.append({
            "xq4": xswiz(q[b].T),
            "xk4": xswiz(k[b].T),
            "xv4": xswiz(v[b].T),
            "wq3": wswiz((Wq[rows] * scale).T),
            "wk3": wswiz(Wk[rows].T),
            "wv3": wswiz(Wv[rows].T),
            "wo3": np.ascontiguousarray(wo3),
            "bcat": np.ascontiguousarray(bcat),
            "msk": masks,
        })
    return in_maps


_CACHE = {}


def _get_nc(causal):
    if causal not in _CACHE:
        _CACHE[causal] = build(S_FULL, causal)
    return _CACHE[causal]


def kernel(query, key, value, Wq, bq, Wk, bk, Wv, bv, Wo, bo, is_causal):
    causal = bool(int(np.asarray(is_causal)))
    nc = _get_nc(causal)
    in_maps = make_in_maps(query, key, value, Wq, bq, Wk, bk, Wv, bv, Wo, bo)
    res = run_bass_kernel_spmd(nc, in_maps, core_ids=list(range(NCORES)))
    out = np.zeros((B, S_FULL, E), np.float32)
    for c in range(NCORES):
        b, tp = divmod(c, TP)
        out[b] += res.results[c]["outT"].T.astype(np.float32)
    out += np.asarray(bo, np.float32)
    return out


# revision 48
# speedup vs baseline: 1.2685x; 1.0236x over previous
"""Multi-head causal attention (QKV proj + attention + out proj) on 8 TRN2
NeuronCores.

Sharding: 2-way data-parallel over batch x 4-way tensor-parallel over heads
(Megatron-style).  Core c handles batch c//4 and heads [4*(c%4), 4*(c%4)+4).
Each core computes its 4 heads' Q/K/V projections (column-parallel), the
attention for those heads, and a partial output projection (row-parallel).
The host sums the 4 TP partials per batch and adds the output bias.

v2 design notes (vs the 333us baseline, which was PE-bound at 1.2 GHz):
  - everything fp16 on the wire: x, weights, output partials are cast on the
    host, halving HBM traffic (30MB -> ~15MB per core).  Host also
    pre-swizzles x/weights into partition-major layout so every input DMA is
    a contiguous per-partition stream.
  - exp batching: scores for 4 k-tiles accumulate into one [128, 2048] PSUM
    region (4 banks) and are exp'd by ONE activation instruction --
    (2048+352)/1.2 = 2.0us per 4 tiles vs 4x720ns.  ACT is the attention
    pace-setter so this matters.
  - normalization: reciprocal_approx_fast (custom DVE op, ~670ns for [1,512])
    replaces the 3355ns iterative reciprocal; the chain is emitted one head
    late so it never blocks the PE stream.
  - software-pipelined emission: the engines are in-order, so the PE
    instruction stream interleaves attention groups of chunk j with
    projection chains of chunk j+1 and out-projection blocks of chunk j-1
    ("filler" units).  The PE never sits on an exp dependency and the HAM
    clock gate stays at 8/8 (2.4 GHz).
"""

import numpy as np
from collections import deque
from contextlib import ExitStack

import concourse.bass as bass
import concourse.mybir as mybir
import concourse.tile as tile
from concourse import bacc
from concourse.bass import ds
from concourse.bass_utils import run_bass_kernel_spmd


B, S_FULL, E, H = 2, 2048, 1024, 16
D = E // H          # 64
NCORES = 8
TP = 4              # tensor-parallel ways (over heads)
HL = H // TP        # 4 local heads per core
F = HL * D          # 256 local projection width
P = 128
QCH = 512           # q-chunk / matmul moving-dim size
GRP = 4             # k-tiles exp'd per activation instruction
FP32 = mybir.dt.float32
F32R = mybir.dt.float32r
F16 = mybir.dt.float16
AF = mybir.ActivationFunctionType


def build(S=S_FULL, causal=True, debug=False):
    ET = E // P          # 8 contraction tiles for projections
    NQ = S // QCH        # 4 q chunks
    KT = S // P          # 16 k tiles
    KPQ = QCH // P       # 4 k tiles per q chunk

    nc = bacc.Bacc()

    def din(name, shape, dt=F16):
        return nc.declare_dram_parameter(name, shape, dt, isOutput=False)

    # host pre-swizzled, all fp16 (see make_in_maps)
    xq4 = din("xq4", [NQ, P, ET, QCH])
    xk4 = din("xk4", [NQ, P, ET, QCH])
    xv4 = din("xv4", [NQ, P, ET, QCH])
    wq3 = din("wq3", [P, ET, F])
    wk3 = din("wk3", [P, ET, F])
    wv3 = din("wv3", [P, ET, F])
    wo3 = din("wo3", [P, F // P, E])
    bcat = din("bcat", [P, 2 + 2 + F], FP32)   # bq2 | bk2 | bvb
    msk = din("msk", [P, KPQ, 2, QCH])
    outT = nc.declare_dram_parameter("outT", [E, S], F16, isOutput=True)
    if debug:
        dbg_qT = nc.declare_dram_parameter("dbg_qT", [P, F // P, S], F16, isOutput=True)
        dbg_kT = nc.declare_dram_parameter("dbg_kT", [P, F // P, S], F16, isOutput=True)
        dbg_vo = nc.declare_dram_parameter("dbg_vo", [P, KT, HL, D + 1], F16, isOutput=True)
        dbg_oT = nc.declare_dram_parameter("dbg_oT", [P, F // P, S], F16, isOutput=True)
        dbg_po = nc.declare_dram_parameter("dbg_po", [P, HL, QCH], FP32, isOutput=True)
        dbg_bc = nc.declare_dram_parameter("dbg_bc", [D, HL, QCH], FP32, isOutput=True)

    with ExitStack() as ctx:
        ctx.enter_context(
            nc.allow_low_precision(reason="fp16 matmuls are the design point")
        )
        tc = ctx.enter_context(tile.TileContext(nc))
        const = ctx.enter_context(tc.tile_pool(name="const", bufs=1))
        xp = ctx.enter_context(tc.tile_pool(name="xp", bufs=2))  # 2 bufs x 3 tags
        ptp = ctx.enter_context(tc.tile_pool(name="ptp", bufs=3))
        dnp = ctx.enter_context(tc.tile_pool(name="dnp", bufs=2))
        otp = ctx.enter_context(tc.tile_pool(name="otp", bufs=3))
        # PSUM: sc 2x2 banks + acc 2 + po 2 = 8
        scp = ctx.enter_context(tc.tile_pool(name="scp", bufs=2, space="PSUM"))
        accp = ctx.enter_context(tc.tile_pool(name="accp", bufs=2, space="PSUM"))
        pop = ctx.enter_context(tc.tile_pool(name="pop", bufs=2, space="PSUM"))

        # ---- constants / persistent tensors ----
        # masks first: the PE warm-up matmuls depend only on this small DMA,
        # so the PE clock ramps while the big loads stream in.
        msk_sb = const.tile([P, KPQ, 2, QCH], F16)
        nc.sync.dma_start(out=msk_sb, in_=msk[:, :, :, :])
        wq_sb = const.tile([P, ET, F], F16)
        nc.sync.dma_start(out=wq_sb, in_=wq3[:, :, :])
        wk_sb = const.tile([P, ET, F], F16)
        nc.sync.dma_start(out=wk_sb, in_=wk3[:, :, :])
        bcat_sb = const.tile([P, 2 + 2 + F], FP32)
        nc.sync.dma_start(out=bcat_sb, in_=bcat[:, :])
        bq_sb = bcat_sb[:, 0:2]
        bk_sb = bcat_sb[:, 2:4]
        bvb_sb = bcat_sb[:, 4:4 + F]
        # wv/wo load AFTER the chunk-0 activations (emitted in the main
        # sequence below): they aren't needed until the V projection /
        # out-projection, and this unblocks proj(0)'s QK chains ~3us earlier.
        wv_sb = const.tile([P, ET, F], F16)
        wo_sb = const.tile([P, F // P, E], F16)

        # PE clock warm-up: back-to-back dummy matmuls (WAW-serialized on the
        # acc pool) keep the tensor engine busy through the HAM window while
        # the input DMAs stream, so real work starts at 2.4 GHz.
        for _ in range(14):
            wps = accp.tile([P, QCH], FP32, tag="acc")
            nc.tensor.matmul(
                wps, msk_sb[:, 0, 0, 0:P], msk_sb[:, 0, 0, :],
                start=True, stop=True,
            )

        ones_f32 = const.tile([P, D], FP32)
        nc.vector.memset(ones_f32, 1.0)
        ones_f16 = const.tile([P, D], F16)
        nc.scalar.activation(ones_f16, ones_f32, AF.Copy)

        qT_sb = const.tile([P, F // P, S], F16)
        kT_sb = const.tile([P, F // P, S], F16)
        # V with a trailing ones column: the PV matmul emits the softmax
        # denominator as PSUM row D for free.
        vo_sb = const.tile([P, KT, HL, D + 1], F16)
        nc.scalar.activation(
            vo_sb[:, :, :, D:D + 1],
            ones_f32[:, 0:KT * HL].rearrange("p (a b c) -> p a b c", a=KT, b=HL, c=1),
            AF.Copy,
        )
        oT_sb = const.tile([P, F // P, S], F16)
        # unnormalized attention output + denominator row (row D), per head
        ou_all = const.tile([P, HL, S], F16)

        # x chunk DMAs (j-granular); emitted early and prefetched one chunk
        # ahead by the main loop.
        x_tiles = {}

        def emit_x_dma(j):
            for name, src in (("q", xq4), ("k", xk4), ("v", xv4)):
                t = xp.tile([P, ET, QCH], F16, tag=f"x{name}")
                nc.sync.dma_start(out=t, in_=src[j])
                x_tiles[(name, j)] = t

        # ---- projection / out-projection unit generators (PE fillers) ----
        def proj_qk_unit(j, which, blk):
            xt = x_tiles[(which, j)]
            w_sb = wq_sb if which == "q" else wk_sb
            b_sb = bq_sb if which == "q" else bk_sb
            dst = qT_sb if which == "q" else kT_sb
            acc = accp.tile([P, QCH], FP32, tag="acc")
            for et in range(ET):
                nc.tensor.matmul(
                    acc,
                    w_sb[:, et, ds(blk * P, P)],
                    xt[:, et, :],
                    start=(et == 0),
                    stop=(et == ET - 1),
                )
            nc.vector.tensor_scalar_add(
                dst[:, blk, ds(j * QCH, QCH)], acc, b_sb[:, blk:blk + 1]
            )
            return 1750

        def proj_v_unit(j, sl):
            xt = x_tiles[("v", j)]
            st = j * KPQ + sl
            acc = accp.tile([P, QCH], FP32, tag="acc")
            for et in range(ET):
                nc.tensor.matmul(
                    acc[:, 0:F],
                    xt[:, et, ds(sl * P, P)],
                    wv_sb[:, et, :],
                    start=(et == 0),
                    stop=(et == ET - 1),
                )
            nc.vector.tensor_add(
                vo_sb[:, st, :, 0:D],
                acc[:, 0:F].rearrange("p (h d) -> p h d", h=HL),
                bvb_sb.rearrange("p (h d) -> p h d", h=HL),
            )
            return 900

        def outproj_unit(j, eb):
            acc = accp.tile([P, QCH], FP32, tag="acc")
            for fb in range(F // P):
                nc.tensor.matmul(
                    acc,
                    wo_sb[:, fb, ds(eb * P, P)],
                    oT_sb[:, fb, ds(j * QCH, QCH)],
                    start=(fb == 0),
                    stop=(fb == F // P - 1),
                )
            ot = otp.tile([P, QCH], F16, tag="ot")
            if eb % 2 == 0:
                nc.vector.tensor_copy(ot, acc)
            else:
                nc.scalar.activation(ot, acc, AF.Copy)
            nc.sync.dma_start(out=outT[ds(eb * P, P), ds(j * QCH, QCH)], in_=ot)
            return 500

        fillers = deque()

        def emit_dummy_mm():
            # keep-warm matmul: occupies an otherwise-idle PE stall window so
            # the HAM clock gate never sees an idle MID window (re-throttle
            # to 1.2 GHz costs far more than the dummy's 320ns).
            wps = accp.tile([P, QCH], FP32, tag="acc")
            nc.tensor.matmul(
                wps, msk_sb[:, 0, 0, 0:P], msk_sb[:, 0, 0, :],
                start=True, stop=True,
            )
            return 320

        def do_filler(budget, pad=700):
            while budget > 0 and fillers:
                budget -= fillers.popleft()()
            pad = min(budget, pad)
            while pad > 0:
                pad -= emit_dummy_mm()

        def drain_fillers():
            while fillers:
                fillers.popleft()()

        def push_proj(j):
            for blk in range(F // P):
                fillers.append(lambda j=j, b=blk: proj_qk_unit(j, "q", b))
                fillers.append(lambda j=j, b=blk: proj_qk_unit(j, "k", b))
            for sl in range(KPQ):
                fillers.append(lambda j=j, s=sl: proj_v_unit(j, s))

        def push_outproj(j):
            for eb in range(E // P):
                fillers.append(lambda j=j, e=eb: outproj_unit(j, e))

        if debug:
            dbg_po_sb = const.tile([P, HL, QCH], FP32)
            dbg_bc_sb = const.tile([D, HL, QCH], FP32)

        # ---- normalization ----
        # po (unnormalized O + denom row) is evacuated to SBUF fp16 right
        # after the last PV; per chunk j, ONE tiny DMA reshapes the 4 heads'
        # denominator rows [1, 4x512] into [128, 16] so a single DVE
        # reciprocal covers them at full lane parallelism (~265ns vs 3.4us
        # per single-partition reciprocal), then a DMA puts 1/denom back as
        # a row for the PE broadcast matmuls.
        def emit_evac(j, h, po_t):
            nc.scalar.activation(
                ou_all[0:D + 1, h, ds(j * QCH, QCH)], po_t[0:D + 1, :], AF.Copy
            )
            if debug and j == 0:
                nc.vector.tensor_copy(dbg_po_sb[:, h, :], po_t)

        def emit_norm_pair(j, pr):
            PPH = QCH // 16  # 32 partitions per head's denominator row
            dn = dnp.tile([2 * PPH, 16], F16, tag="dn")
            for i in range(2):
                nc.sync.dma_start(
                    out=dn[i * PPH:(i + 1) * PPH, :],
                    in_=ou_all[D:D + 1, 2 * pr + i, ds(j * QCH, QCH)],
                )
            rc = dnp.tile([2 * PPH, 16], F16, tag="rc")
            nc.vector.reciprocal(rc, dn)
            rcr = dnp.tile([1, 2, QCH], F16, tag="rcr")
            for i in range(2):
                nc.sync.dma_start(
                    out=rcr[:, i, :], in_=rc[i * PPH:(i + 1) * PPH, :]
                )
            for i in range(2):
                h = 2 * pr + i
                doff = i * D
                bc = accp.tile([P, QCH], FP32, tag="acc")
                nc.tensor.matmul(
                    bc[0:D, :], ones_f16[0:1, :], rcr[:, i, :],
                    start=True, stop=True,
                )
                if debug and j == 0:
                    nc.vector.tensor_copy(dbg_bc_sb[:, h, :], bc[0:D, :])
                nc.vector.tensor_mul(
                    oT_sb[doff:doff + D, pr, ds(j * QCH, QCH)],
                    ou_all[0:D, h, ds(j * QCH, QCH)],
                    bc[0:D, :],
                )

        # ---- main emission loop ----
        emit_x_dma(0)
        nc.sync.dma_start(out=wv_sb, in_=wv3[:, :, :])
        nc.sync.dma_start(out=wo_sb, in_=wo3[:, :, :])
        push_proj(0)
        drain_fillers()          # projections for chunk 0 up front

        pending = None
        for j in range(NQ):
            if j + 1 < NQ:
                emit_x_dma(j + 1)
                push_proj(j + 1)
            for pr in range(HL // 2):
                # head pair (hA, hB) = (2*pr, 2*pr+1): hA's Q/K live on
                # partitions 0-63 of block pr, hB's on 64-127.  Their QK^T
                # matmuls (64-row contraction each) are emitted back-to-back
                # with explicit tile_position so they stream CONCURRENTLY
                # through disjoint PE row groups -- ~2x scores throughput.
                hA, hB = 2 * pr, 2 * pr + 1
                nkt = KPQ * (j + 1) if causal else KT
                po_a = pop.tile([P, QCH], FP32, tag="po")
                po_b = pop.tile([P, QCH], FP32, tag="po")
                for kt in range(nkt):
                    # one k-tile for both heads per group; sc is [128,2,512]
                    # (2 banks) double-buffered so the next group's scores
                    # never wait on this group's exp.
                    sc = scp.tile([P, 2, QCH], FP32, tag="sc")
                    nc.tensor.matmul(
                        sc[:, 0, :],
                        kT_sb[0:D, pr, ds(kt * P, P)],
                        qT_sb[0:D, pr, ds(j * QCH, QCH)],
                        start=True, stop=True,
                        tile_position=(0, 0),
                    )
                    nc.tensor.matmul(
                        sc[:, 1, :],
                        kT_sb[D:P, pr, ds(kt * P, P)],
                        qT_sb[D:P, pr, ds(j * QCH, QCH)],
                        start=True, stop=True,
                        tile_position=(64, 0),
                    )
                    pt = ptp.tile([P, 2, QCH], F16, tag="pt")
                    if j >= 2 and kt % 3 == 0:
                        # late chunks are ACT-bound: alternate exp groups onto
                        # the DVE via the fp16 bit-trick
                        # exp(x) ~= bitcast_f16(int16(x*1024/ln2 + 15360)).
                        # Valid for x in (-10.4, 10.6); scores here are ~+-3.
                        nc.vector.tensor_scalar(
                            pt.bitcast(mybir.dt.int16),
                            sc,
                            1477.3194,
                            15360.0,
                            op0=mybir.AluOpType.mult,
                            op1=mybir.AluOpType.add,
                        )
                    else:
                        nc.scalar.activation(pt, sc, AF.Exp)
                    if causal and kt >= KPQ * j:
                        # diagonal k-tile: zero the upper-triangular part
                        t = kt - KPQ * j
                        nc.vector.tensor_mul(
                            pt, pt, msk_sb[:, t, :, :],
                        )
                    do_filler(700)
                    nc.tensor.matmul(
                        po_a[0:D + 1, :],
                        vo_sb[:, kt, hA, :],
                        pt[:, 0, :],
                        start=(kt == 0),
                        stop=(kt == nkt - 1),
                    )
                    nc.tensor.matmul(
                        po_b[0:D + 1, :],
                        vo_sb[:, kt, hB, :],
                        pt[:, 1, :],
                        start=(kt == 0),
                        stop=(kt == nkt - 1),
                    )
                emit_evac(j, hA, po_a)
                emit_evac(j, hB, po_b)
                if pending is not None:
                    pj, ppr = pending
                    emit_norm_pair(pj, ppr)
                    if ppr == 1:
                        push_outproj(pj)
                    pending = None
                pending = (j, pr)
            # chunk boundary: everything for chunk j+1's attention must be
            # emitted before its first scores matmul.
            drain_fillers()
        emit_norm_pair(*pending)
        push_outproj(NQ - 1)
        drain_fillers()
        if debug:
            nc.sync.dma_start(out=dbg_qT[:, :, :], in_=qT_sb)
            nc.sync.dma_start(out=dbg_kT[:, :, :], in_=kT_sb)
            nc.sync.dma_start(out=dbg_vo[:, :, :, :], in_=vo_sb)
            nc.sync.dma_start(out=dbg_oT[:, :, :], in_=oT_sb)
            nc.sync.dma_start(out=dbg_po[:, :, :], in_=dbg_po_sb)
            nc.sync.dma_start(out=dbg_bc[:, :, :], in_=dbg_bc_sb)

    nc.compile()
    return nc


def make_masks(S=S_FULL):
    KPQ = QCH // P
    m = np.zeros((P, KPQ, QCH), np.float32)
    for t in range(KPQ):
        kk = np.arange(P)[:, None]
        qq = np.arange(QCH)[None, :]
        m[:, t, :] = (qq >= kk + P * t).astype(np.float32)
    return m


def make_in_maps(query, key, value, Wq, bq, Wk, bk, Wv, bv, Wo, bo, S=S_FULL):
    scale = float(D) ** -0.5
    ET = E // P
    NQ = S // QCH
    q = np.asarray(query, np.float32)
    k = np.asarray(key, np.float32)
    v = np.asarray(value, np.float32)
    Wq = np.asarray(Wq, np.float32)
    Wk = np.asarray(Wk, np.float32)
    Wv = np.asarray(Wv, np.float32)
    Wo = np.asarray(Wo, np.float32)
    bq = np.asarray(bq, np.float32)
    bk = np.asarray(bk, np.float32)
    bv = np.asarray(bv, np.float32)

    def xswiz(xT):
        # [E, S] -> [NQ, P, ET, QCH]: contiguous per-partition DMA streams
        return np.ascontiguousarray(
            xT.reshape(ET, P, NQ, QCH).transpose(2, 1, 0, 3).astype(np.float16)
        )

    def wswiz(wT):
        # [E, F] -> [P, ET, F]
        return np.ascontiguousarray(
            wT.reshape(ET, P, F).transpose(1, 0, 2).astype(np.float16)
        )

    masks = make_masks(S)
    msk8 = np.ascontiguousarray(
        np.broadcast_to(masks[:, :, None, :], (P, QCH // P, 2, QCH))
    ).astype(np.float16)
    in_maps = []
    for c in range(NCORES):
        b, tp = divmod(c, TP)
        rows = slice(tp * F, (tp + 1) * F)
        bq2 = (bq[rows] * scale).reshape(F // P, P).T        # [P, 2]
        bk2 = bk[rows].reshape(F // P, P).T                  # [P, 2]
        bvb = np.broadcast_to(bv[rows], (P, F))              # [P, F]
        bcat = np.concatenate([bq2, bk2, bvb], axis=1).astype(np.float32)
        woT = Wo[:, rows].T                                  # [F, E]
        wo3 = woT.reshape(F // P, P, E).transpose(1, 0, 2).astype(np.float16)
        in_maps.append({
            "xq4": xswiz(q[b].T),
            "xk4": xswiz(k[b].T),
            "xv4": xswiz(v[b].T),
            "wq3": wswiz((Wq[rows] * scale).T),
            "wk3": wswiz(Wk[rows].T),
            "wv3": wswiz(Wv[rows].T),
            "wo3": np.ascontiguousarray(wo3),
            "bcat": np.ascontiguousarray(bcat),
            "msk": msk8,
        })
    return in_maps


_CACHE = {}


def _get_nc(causal):
    if causal not in _CACHE:
        _CACHE[causal] = build(S_FULL, causal)
    return _CACHE[causal]


def kernel(query, key, value, Wq, bq, Wk, bk, Wv, bv, Wo, bo, is_causal):
    causal = bool(int(np.asarray(is_causal)))
    nc = _get_nc(causal)
    in_maps = make_in_maps(query, key, value, Wq, bq, Wk, bk, Wv, bv, Wo, bo)
    res = run_bass_kernel_spmd(nc, in_maps, core_ids=list(range(NCORES)))
    out = np.zeros((B, S_FULL, E), np.float32)
    for c in range(NCORES):
        b, tp = divmod(c, TP)
        out[b] += res.results[c]["outT"].T.astype(np.float32)
    out += np.asarray(bo, np.float32)
    return out
